# revision 15
# baseline (speedup 1.0000x reference)
"""Multi-head attention (B=2, S=2048, D=1024, H=16) on 8 TRN2 NeuronCores.

Sharding: core c handles batch c//4 and head-group c%4 (4 heads each).
Host pre-transposes inputs/weights to d-major bf16; each core computes
its 4 heads' projections, causal attention, and a partial (row-parallel)
dense output [S, D] which the host sums across the 4 cores of each batch.

Structural design (all matmuls bf16 - fp8 DoubleRow was measured to
give zero per-column speedup on this hardware):

- Scores are computed transposed ([k, q] layout) with K zero-padded to
  128 so the PE activity monitor stays unthrottled; no on-chip
  transposes anywhere.
- The V tiles carry a leading ones-column (then 63 zeros, then the 64 v
  columns), so the softmax row-sum lands on PSUM PARTITION 0 and av on
  partitions 64:128: the reciprocal runs directly on partition 0 and
  gpsimd.partition_broadcast fans it across partitions - no DRAM
  round-trip anywhere in the normalize chain.
- qt/kt zero-padding halves and vh1 constants are initialized by
  vector/gpsimd memsets split across both engines at startup.
- Emission interleaves next-chunk projection pieces and prev-chunk dense
  pieces between attention score/AV groups ("fillers") so the in-order
  PE queue always has independent work while exp (scalar) catches up.
- Masking is applied as a multiplicative factor on the exp'd scores;
  fully-masked 128-col blocks are never computed, partially-masked ones
  are trimmed at emit time from the actual mask contents.
"""

import numpy as np
import ml_dtypes

import concourse.bass as bass
import concourse.tile as tile
from concourse import bacc, mybir
from concourse.bass_utils import run_bass_kernel_spmd

BF16 = mybir.dt.bfloat16
F32 = mybir.dt.float32
FP8 = mybir.dt.float8e4
NPBF16 = ml_dtypes.bfloat16
NPFP8 = ml_dtypes.float8_e4m3

D_MODEL = 1024
NH = 16
DEPTH = 64
B = 2
S = 2048
N_CORES = 8
GROUPS = 4              # head-groups (tensor parallel dimension)
HPG = NH // GROUPS      # 4 heads per core
OG = HPG * DEPTH        # 256 projection output cols per core
QC = 512                # q chunk (matmul free dim)
NQC = S // QC           # 4
KT = 128                # k tile (psum partition dim)
NKT = S // KT           # 16
DK = D_MODEL // 128     # 8 contraction tiles of 128
SC = 512                # projection s chunk
NSC = S // SC           # 4
EGRP = 1                # k-tiles per exp group (psum group tile)
HORDER = (0, 2, 1, 3)   # even heads first: their avf partition-shift DMAs
                        # start earlier and hide under the odd heads' attn

TRACE = False
TRACE_KW = {}
LAST_RESULT = None
_CACHE = {}


def _chunk(lst, n):
    return [lst[i : i + n] for i in range(0, len(lst), n)]


def _build(ktiles, mk_w, zero_bias, num_devices=N_CORES, sim_init=False):
    """Emit the bass program. ktiles[j] = [(t, lo, tri), ...] computed
    k-tiles for q-chunk j (see _classify_mask)."""
    exp_scale = 0.125
    nc = bacc.Bacc(
        "TRN2", target_bir_lowering=False, debug=False, num_devices=num_devices
    )
    xq = nc.dram_tensor("xq", [NSC, 128, DK, SC], BF16, kind="ExternalInput").ap()
    xk = nc.dram_tensor("xk", [NSC, 128, DK, SC], BF16, kind="ExternalInput").ap()
    xv = nc.dram_tensor("xv", [NSC, 128, DK, SC], BF16, kind="ExternalInput").ap()
    wq = nc.dram_tensor("wq", [128, DK, OG], BF16, kind="ExternalInput").ap()
    wk = nc.dram_tensor("wk", [128, DK, OG], BF16, kind="ExternalInput").ap()
    wv = nc.dram_tensor("wv", [128, DK, OG], BF16, kind="ExternalInput").ap()
    wd = nc.dram_tensor("wd", [128, 2, D_MODEL], BF16, kind="ExternalInput").ap()
    # biases pre-masked per (parity, oc); only used when nonzero
    pb_q = nc.dram_tensor("pb_q", [128, 2, 2], F32, kind="ExternalInput").ap()
    pb_k = nc.dram_tensor("pb_k", [128, 2, 2], F32, kind="ExternalInput").ap()
    mk = nc.dram_tensor("mk", [128, mk_w], BF16, kind="ExternalInput").ap()
    outp = nc.dram_tensor("outp", [S, D_MODEL], BF16, kind="ExternalOutput").ap()

    Exp = mybir.ActivationFunctionType.Exp

    with tile.TileContext(nc) as tc:
        with (
            tc.tile_pool(name="singles", bufs=1) as singles,
            tc.tile_pool(name="xin", bufs=6) as xin,
            tc.tile_pool(name="exps", bufs=4) as exps,
            tc.tile_pool(name="small", bufs=4) as small,
            tc.tile_pool(name="bcastp", bufs=4) as bcastp,
            tc.tile_pool(name="ost", bufs=3) as ost,
            # PSUM: scores groups (2 banks x 2 bufs) + shared pool for
            # projections / AV accumulation / dense (1 bank x 4 bufs)
            tc.tile_pool(name="psc", bufs=4, space="PSUM") as psc,
            tc.tile_pool(name="ppav", bufs=4, space="PSUM") as ppav,
        ):
            wq_sb = singles.tile([128, DK, OG], BF16)
            nc.sync.dma_start(wq_sb[:], wq)
            # per-partition parity masks ([1]*64+[0]*64 and its complement):
            # multiplying the full-height psum by a mask column writes the
            # head data AND its zero padding in one op. Built by four tiny
            # memsets so no DMA gates the chunk-0 casts.
            pm_sb = singles.tile([128, 2], F32)
            nc.vector.memset(pm_sb[0:64, 0:1], 1.0)
            nc.vector.memset(pm_sb[64:128, 0:1], 0.0)
            nc.vector.memset(pm_sb[0:64, 1:2], 0.0)
            nc.vector.memset(pm_sb[64:128, 1:2], 1.0)
            if not zero_bias:
                pbq_sb = singles.tile([128, 2, 2], F32)
                nc.sync.dma_start(pbq_sb[:], pb_q)
                pbk_sb = singles.tile([128, 2, 2], F32)
                nc.sync.dma_start(pbk_sb[:], pb_k)
            else:
                pbq_sb = pbk_sb = None
            wd_sb = singles.tile([128, 2, D_MODEL], BF16)  # loaded after sc=0 DMAs

            # per-head layouts, zero-padded to K=128 (keeps the PE's HAM
            # activity monitor warm; K=64 streams never unthrottle).
            # head h occupies d-rows [(h%2)*64, (h%2)*64+64); rest zeros.
            qt = singles.tile([128, HPG, S], BF16)
            kt_ = singles.tile([128, HPG, S], BF16)
            # [p = k%128, ktile, head, ones col + 63 zeros + 64 v cols]
            vh1 = singles.tile([128, NKT, HPG, 128], BF16)
            avb = singles.tile([128, 2, S], BF16)   # normalized av^T

            # vh1 cols 1:64 are never initialized: they only feed psum rows
            # 1:64 of the AV output, which nothing reads. qt/kt padding is
            # written by the parity-masked projection casts, so the only
            # startup init is this tiny ones-column memset.
            nc.gpsimd.memset(vh1[:, :, :, 0:1], 1.0)
            if sim_init:
                # CoreSim rejects reads of uninitialized SBUF; on hardware
                # these columns only feed psum rows nothing reads
                nc.gpsimd.memset(vh1[:, :, :, 1:64], 0.0)

            x_tiles = {}

            def emit_chunk_dmas(sc, keys=("q", "k", "v")):
                for key in keys:
                    src = {"q": xq, "k": xk, "v": xv}[key]
                    x_sb = xin.tile([128, DK, SC], BF16, tag="xin")
                    # two half-DMAs so the first projection matmuls start
                    # as soon as the dk 0:4 half lands
                    nc.sync.dma_start(x_sb[:, 0 : DK // 2, :], src[sc, :, 0 : DK // 2])
                    nc.sync.dma_start(x_sb[:, DK // 2 :, :], src[sc, :, DK // 2 :])
                    x_tiles[(sc, key)] = x_sb

            def emit_qk_piece(sc, which, oc):
                """One oc-half (128 output cols = 2 head-halves) of the q or
                k projection for s-chunk sc, written into qt/kt_."""
                x_sb = x_tiles[(sc, which)]
                w_sb = wq_sb if which == "q" else wk_sb
                dst = qt if which == "q" else kt_
                ssl = slice(sc * SC, (sc + 1) * SC)
                ps = ppav.tile([128, SC], F32, tag="ppav")
                for dk in range(DK):
                    nc.tensor.matmul(
                        ps[:],
                        lhsT=w_sb[:, dk, oc * 128 : (oc + 1) * 128],
                        rhs=x_sb[:, dk, :],
                        start=(dk == 0),
                        stop=(dk == DK - 1),
                    )
                pb_sb = pbq_sb if which == "q" else pbk_sb
                for half in range(2):
                    ch = 2 * oc + half
                    if zero_bias:
                        nc.vector.tensor_scalar(
                            out=dst[:, ch, ssl],
                            in0=ps[:],
                            scalar1=pm_sb[:, half : half + 1],
                            scalar2=None,
                            op0=mybir.AluOpType.mult,
                        )
                    else:
                        nc.vector.tensor_scalar(
                            out=dst[:, ch, ssl],
                            in0=ps[:],
                            scalar1=pm_sb[:, half : half + 1],
                            scalar2=pb_sb[:, half, oc : oc + 1],
                            op0=mybir.AluOpType.mult,
                            op1=mybir.AluOpType.add,
                        )

            def emit_v_piece(sc, sth):
                """One 128-row k-tile of the v projection -> vh1."""
                x_sb = x_tiles[(sc, "v")]
                st = sc * (SC // KT) + sth
                ps = ppav.tile([128, SC], F32, tag="ppav")
                ksl = slice(sth * KT, (sth + 1) * KT)
                for dk in range(DK):
                    nc.tensor.matmul(
                        ps[:, :OG],
                        lhsT=x_sb[:, dk, ksl],
                        rhs=wv_sb[:, dk, :],
                        start=(dk == 0),
                        stop=(dk == DK - 1),
                    )
                nc.vector.tensor_copy(
                    out=vh1[:, st, :, 64:128],
                    in_=ps[:, :OG].rearrange("p (h d) -> p h d", d=DEPTH),
                )

            pending_ot = {}

            def emit_dense_piece(st, oc):
                """Half (512 out cols) of a 128-row dense output block."""
                if oc == 0:
                    pending_ot[st] = ost.tile(
                        [128, D_MODEL], BF16, tag="ostage", name=f"ot{st}"
                    )
                ot = pending_ot[st]
                ps = ppav.tile([128, SC], F32, tag="ppav")
                for co in range(2):
                    nc.tensor.matmul(
                        ps[:],
                        lhsT=avb[:, co, st * 128 : (st + 1) * 128],
                        rhs=wd_sb[:, co, oc * 512 : (oc + 1) * 512],
                        start=(co == 0),
                        stop=(co == 1),
                    )
                if oc == 0:
                    nc.vector.tensor_copy(
                        out=ot[:, oc * 512 : (oc + 1) * 512], in_=ps[:]
                    )
                else:
                    nc.scalar.copy(out=ot[:, oc * 512 : (oc + 1) * 512], in_=ps[:])
                    nc.sync.dma_start(outp[st * 128 : (st + 1) * 128, :], ot[:])
                    del pending_ot[st]

            def emit_attn(h, j, fillers, spacing=1, gctr=None):
                """Causal attention for head h, q-chunk j, popping one filler
                every `spacing` score/exp/AV groups so the independent PE
                work lasts through the whole chunk (late groups are where
                exp throughput limits the pipeline)."""
                if gctr is None:
                    gctr = [0]
                odd = h % 2
                ch = h // 2
                jsl = slice(j * QC, (j + 1) * QC)
                tiles = ktiles[j]
                first, last = tiles[0][0], tiles[-1][0]
                ps_av = ppav.tile([128, QC], F32, tag="ppav")
                if len(tiles) > 1:
                    # 1-tile first group primes the scores->exp->av pipeline
                    groups = [tiles[:1]] + _chunk(tiles[1:], EGRP)
                else:
                    groups = [tiles]
                for grp in groups:
                    ps_g = psc.tile([128, EGRP, QC], F32, tag="psc")
                    for r, (t, lo, runs) in enumerate(grp):
                        # cols [0, lo*128) are fully masked: never computed,
                        # never read by the av matmul below
                        nc.tensor.matmul(
                            ps_g[:, r, lo * 128 :],
                            lhsT=kt_[:, h, t * KT : (t + 1) * KT],
                            rhs=qt[:, h, j * QC + lo * 128 : (j + 1) * QC],
                            start=True,
                            stop=True,
                        )
                    ex = exps.tile([128, EGRP, QC], BF16, tag="exps")
                    if all(lo == 0 for (t, lo, runs) in grp):
                        nc.scalar.activation(
                            out=ex[:, : len(grp), :],
                            in_=ps_g[:, : len(grp), :],
                            func=Exp,
                            scale=0.125,
                        )
                    else:
                        # diagonal tiles: exp only the computed column range
                        for r, (t, lo, runs) in enumerate(grp):
                            nc.scalar.activation(
                                out=ex[:, r, lo * 128 :],
                                in_=ps_g[:, r, lo * 128 :],
                                func=Exp,
                                scale=0.125,
                            )
                    for r, (t, lo, runs) in enumerate(grp):
                        for i0, w, off in runs:
                            nc.vector.tensor_mul(
                                ex[:, r, i0 * 128 : (i0 + w) * 128],
                                ex[:, r, i0 * 128 : (i0 + w) * 128],
                                mk_sb[:, off : off + w * 128],
                            )
                    for r, (t, lo, runs) in enumerate(grp):
                        nc.tensor.matmul(
                            ps_av[:, lo * 128 :],
                            lhsT=vh1[:, t, h, :],
                            rhs=ex[:, r, lo * 128 :],
                            start=(t == first),
                            stop=(t == last),
                        )
                    gctr[0] += 1
                    if fillers and gctr[0] % spacing == 0:
                        fillers.pop(0)()
                # softmax denominator: ones-column -> psum PARTITION 0 ->
                # reciprocal straight out of psum -> gpsimd partition
                # broadcast. No DMA, no staging copy.
                rec0 = small.tile([1, QC], F32, tag="rec0")
                nc.vector.reciprocal_approx_fast(rec0[:], ps_av[0:1, :])
                bc = bcastp.tile([128, QC], F32, tag="bc")
                nc.gpsimd.partition_broadcast(bc[:], rec0[:])
                # av rows live on psum partitions 64:128: normalize straight
                # out of psum into bf16; even heads bounce the normalized
                # bf16 tile through a DMA partition shift into rows 0:64
                if odd:
                    nc.vector.tensor_mul(
                        avb[64:128, ch, jsl], ps_av[64:128, :], bc[64:128, :]
                    )
                else:
                    tmp = bcastp.tile([64, QC], BF16, tag="avtmp")
                    nc.vector.tensor_mul(
                        tmp[:], ps_av[64:128, :], bc[64:128, :]
                    )
                    nc.sync.dma_start(avb[0:64, ch, jsl], tmp[:])

            # ---- interleaved emission ----
            # first-use-ordered DMA issue: xq + wq go first so the first
            # projection matmuls start as early as possible
            emit_chunk_dmas(0, keys=("q",))
            emit_qk_piece(0, "q", 0)
            emit_qk_piece(0, "q", 1)
            wk_sb = singles.tile([128, DK, OG], BF16)
            nc.sync.dma_start(wk_sb[:], wk)
            emit_chunk_dmas(0, keys=("k",))
            emit_qk_piece(0, "k", 0)
            emit_qk_piece(0, "k", 1)
            wv_sb = singles.tile([128, DK, OG], BF16)
            nc.sync.dma_start(wv_sb[:], wv)
            emit_chunk_dmas(0, keys=("v",))
            mk_sb = singles.tile([128, mk_w], BF16)
            nc.sync.dma_start(mk_sb[:], mk)
            for sth in range(SC // KT):
                emit_v_piece(0, sth)
            nc.sync.dma_start(wd_sb[:], wd)  # dense-weight prefetch

            carry = []
            for sc in range(NSC):
                fillers = []
                if sc + 1 < NSC:
                    emit_chunk_dmas(sc + 1)
                    for oc in range(2):
                        fillers.append(
                            lambda sc=sc, oc=oc: emit_qk_piece(sc + 1, "q", oc)
                        )
                        fillers.append(
                            lambda sc=sc, oc=oc: emit_qk_piece(sc + 1, "k", oc)
                        )
                    for sth in range(SC // KT):
                        fillers.append(
                            lambda sc=sc, sth=sth: emit_v_piece(sc + 1, sth)
                        )
                dense = [
                    (lambda st=st, oc=oc: emit_dense_piece(st, oc))
                    for st in range((sc - 1) * 4, sc * 4) if sc >= 1
                    for oc in range(2)
                ]
                # dense pieces have no deadline: defer half of each chunk's
                # so the final chunk (which has no projection fillers left)
                # still has independent PE work between attention groups
                fillers = fillers + carry + (dense[:4] if sc + 1 < NSC else dense)
                carry = dense[4:] if sc + 1 < NSC else []
                ngroups = 4 * max(1, len(ktiles[sc]))
                spacing = max(1, ngroups // max(1, len(fillers)))
                gctr = [0]
                for h in HORDER:
                    emit_attn(h, sc, fillers, spacing, gctr)
                if sc + 1 == NSC:
                    for f in fillers:
                        f()
                else:
                    # leftover projection pieces must land before the next
                    # chunk's attention; dense leftovers join the carry
                    for f in fillers:
                        f()
            for st in range((NSC - 1) * 4, NKT):
                for oc in range(2):
                    emit_dense_piece(st, oc)

    nc.compile()
    return nc


def _classify_mask(mask):
    """Classify 128(k) x 128(q) score blocks from the actual mask contents.

    Returns (ktiles, mk_arr):
      ktiles[j]: list of (t, lo, runs) per computed k-tile for q-chunk j:
        lo: first kept 128-col block within the 512-wide q-chunk (cols
            [0, lo*128) are fully masked and simply never computed/read)
        runs: [(i0, w, off), ...] maximal runs of w consecutive 128-col
            blocks needing a factor multiply; their concatenated factor
            tile lives at mk_arr[:, off : off + w*128]
      mk_arr: [128, W] bf16 multiplicative factors exp(-1e9*m/8)
    """
    m2 = np.asarray(mask, dtype=np.float32).reshape(S, S)
    F = np.exp(m2 * np.float32(-1.25e8))  # exp(-1e9*m/8); 0/1 masks -> 0/1
    if (F.max(axis=1) == 0.0).any():
        raise RuntimeError("mask has fully-masked rows; unsupported")
    blocks = F.reshape(NKT, 128, NKT, 128)  # [qi, qr, t, kr]
    kept = (blocks == 1.0).all(axis=(1, 3))  # [qi, t]
    skip = (blocks == 0.0).all(axis=(1, 3))

    NB = QC // 128  # 128-col blocks per q-chunk
    ktiles = []
    uniq = {}
    chunks = []
    width = [0]

    def run_off(j, t, i0, w):
        fb = np.ascontiguousarray(
            F[(j * NB + i0) * 128 : (j * NB + i0 + w) * 128,
              t * KT : (t + 1) * KT].T
        ).astype(NPBF16)  # [128 k, w*128 q]
        key = fb.tobytes()
        if key not in uniq:
            uniq[key] = width[0]
            chunks.append(fb)
            width[0] += w * 128
        return uniq[key]

    for j in range(NQC):
        qis = list(range(j * NB, (j + 1) * NB))
        tl = []
        for t in range(NKT):
            stats = [
                "k" if kept[qi, t] else ("s" if skip[qi, t] else "m")
                for qi in qis
            ]
            if all(s == "s" for s in stats):
                continue
            lo = next(i for i, s in enumerate(stats) if s != "s")
            need = [i for i in range(lo, NB) if stats[i] != "k"]
            tl.append((t, lo, need))
        if not tl:
            raise RuntimeError("q-chunk with no kept k-tiles; unsupported")
        # the first computed tile must span the full chunk (av 'start' MM)
        if tl[0][1] != 0:
            t0, lo0, need0 = tl[0]
            tl[0] = (t0, 0, sorted(set(need0) | set(range(lo0))))
        # merge consecutive blocks into single wide factor multiplies
        tl2 = []
        for t, lo, need in tl:
            runs = []
            for i in need:
                if runs and runs[-1][0] + runs[-1][1] == i:
                    runs[-1] = (runs[-1][0], runs[-1][1] + 1)
                else:
                    runs.append((i, 1))
            tl2.append(
                (t, lo, [(i0, w, run_off(j, t, i0, w)) for i0, w in runs])
            )
        ktiles.append(tl2)
    if not chunks:
        chunks.append(np.ones((128, KT), dtype=NPBF16))
        width[0] = KT
    mk_arr = np.ascontiguousarray(np.concatenate(chunks, axis=1))
    return ktiles, mk_arr


def _xt_prep(x):
    """[S, D] f32 -> [NSC, 128, DK, SC] bf16, d-major, contiguous S-quarters."""
    xt = x.T.astype(NPBF16)  # [D, S]
    a = xt.reshape(DK, 128, NSC, SC).transpose(2, 1, 0, 3)
    return np.ascontiguousarray(a)


def kernel(v, k, q, mask, wq_w, wq_b, wk_w, wk_b, wv_w, wv_b, dense_w, dense_b):
    global LAST_RESULT
    v = np.asarray(v, dtype=np.float32)
    k = np.asarray(k, dtype=np.float32)
    q = np.asarray(q, dtype=np.float32)
    mask = np.asarray(mask, dtype=np.float32)
    wq_w = np.asarray(wq_w, dtype=np.float32)
    wk_w = np.asarray(wk_w, dtype=np.float32)
    wv_w = np.asarray(wv_w, dtype=np.float32)
    dense_w = np.asarray(dense_w, dtype=np.float32)
    wq_b = np.asarray(wq_b, dtype=np.float32)
    wk_b = np.asarray(wk_b, dtype=np.float32)
    wv_b = np.asarray(wv_b, dtype=np.float32)
    dense_b = np.asarray(dense_b, dtype=np.float32)

    ktiles, mk_arr = _classify_mask(mask)
    zero_bias = not (np.any(wq_b) or np.any(wk_b))
    key = (
        tuple(tuple((t, lo, tuple(rn)) for t, lo, rn in tl) for tl in ktiles),
        mk_arr.shape[1],
        zero_bias,
    )
    if key not in _CACHE:
        _CACHE[key] = _build(ktiles, mk_arr.shape[1], zero_bias)
    nc = _CACHE[key]

    # per-batch inputs (shared by the 4 cores of each batch)
    xq_b = [_xt_prep(q[b]) for b in range(B)]
    xk_b = [_xt_prep(k[b]) for b in range(B)]
    xv_b = [_xt_prep(v[b]) for b in range(B)]

    # per-group weights
    def wslice(w, g):
        ws = w[g * OG : (g + 1) * OG, :].T.astype(NPBF16)  # [D, OG]
        return np.ascontiguousarray(ws.reshape(DK, 128, OG).transpose(1, 0, 2))

    def bslice(b_, g):
        # [128, 2(parity), 2(oc)], zero on the complementary parity rows
        a = b_[g * OG : (g + 1) * OG].astype(np.float32).reshape(2, 2, 64)
        out = np.zeros((128, 2, 2), np.float32)
        for oc in range(2):
            out[0:64, 0, oc] = a[oc, 0]
            out[64:128, 1, oc] = a[oc, 1]
        return out

    wq_g = [wslice(wq_w, g) for g in range(GROUPS)]
    wk_g = [wslice(wk_w, g) for g in range(GROUPS)]
    wv_g = [wslice(wv_w, g) for g in range(GROUPS)]
    qb_g = [bslice(wq_b, g) for g in range(GROUPS)]
    kb_g = [bslice(wk_b, g) for g in range(GROUPS)]
    wd_g = []
    for g in range(GROUPS):
        ds = dense_w[:, g * OG : (g + 1) * OG].T.astype(NPBF16)  # [OG, D]
        wd_g.append(
            np.ascontiguousarray(ds.reshape(2, 128, D_MODEL).transpose(1, 0, 2))
        )

    in_maps = []
    for c in range(N_CORES):
        b, g = c // GROUPS, c % GROUPS
        in_maps.append(
            {
                "xq": xq_b[b],
                "xk": xk_b[b],
                "xv": xv_b[b],
                "wq": wq_g[g],
                "wk": wk_g[g],
                "wv": wv_g[g],
                "wd": wd_g[g],
                "mk": mk_arr,
                "pb_q": qb_g[g],
                "pb_k": kb_g[g],
            }
        )

    kw = dict(trace=True, **TRACE_KW) if TRACE else {}
    res = run_bass_kernel_spmd(nc, in_maps, core_ids=list(range(N_CORES)), **kw)
    LAST_RESULT = res

    corr = dense_w @ wv_b + dense_b  # v-bias pushed through dense, + dense bias
    out = np.empty((B, S, D_MODEL), dtype=np.float32)
    for b in range(B):
        acc = np.zeros((S, D_MODEL), dtype=np.float32)
        for g in range(GROUPS):
            acc += res.results[b * GROUPS + g]["outp"].astype(np.float32)
        out[b] = acc + corr
    return out


# revision 16
# speedup vs baseline: 1.0236x; 1.0236x over previous
"""Multi-head attention (B=2, S=2048, D=1024, H=16) on 8 TRN2 NeuronCores.

Sharding: core c handles batch c//4 and head-group c%4 (4 heads each).
Host pre-transposes inputs/weights to d-major bf16; each core computes
its 4 heads' projections, causal attention, and a partial (row-parallel)
dense output [S, D] which the host sums across the 4 cores of each batch.

Structural design (all matmuls bf16 - fp8 DoubleRow was measured to
give zero per-column speedup on this hardware):

- Scores are computed transposed ([k, q] layout) with K zero-padded to
  128 so the PE activity monitor stays unthrottled; no on-chip
  transposes anywhere.
- The V tiles carry a leading ones-column (then 63 zeros, then the 64 v
  columns), so the softmax row-sum lands on PSUM PARTITION 0 and av on
  partitions 64:128: the reciprocal runs directly on partition 0 and
  gpsimd.partition_broadcast fans it across partitions - no DRAM
  round-trip anywhere in the normalize chain.
- qt/kt zero-padding halves and vh1 constants are initialized by
  vector/gpsimd memsets split across both engines at startup.
- Emission interleaves next-chunk projection pieces and prev-chunk dense
  pieces between attention score/AV groups ("fillers") so the in-order
  PE queue always has independent work while exp (scalar) catches up.
- Masking is applied as a multiplicative factor on the exp'd scores;
  fully-masked 128-col blocks are never computed, partially-masked ones
  are trimmed at emit time from the actual mask contents.
"""

import numpy as np
import ml_dtypes

import concourse.bass as bass
import concourse.tile as tile
from concourse import bacc, mybir
from concourse.bass_utils import run_bass_kernel_spmd

BF16 = mybir.dt.bfloat16
F32 = mybir.dt.float32
FP8 = mybir.dt.float8e4
NPBF16 = ml_dtypes.bfloat16
NPFP8 = ml_dtypes.float8_e4m3

D_MODEL = 1024
NH = 16
DEPTH = 64
B = 2
S = 2048
N_CORES = 8
GROUPS = 4              # head-groups (tensor parallel dimension)
HPG = NH // GROUPS      # 4 heads per core
OG = HPG * DEPTH        # 256 projection output cols per core
QC = 512                # q chunk (matmul free dim)
NQC = S // QC           # 4
KT = 128                # k tile (psum partition dim)
NKT = S // KT           # 16
DK = D_MODEL // 128     # 8 contraction tiles of 128
SC = 512                # projection s chunk
NSC = S // SC           # 4
EGRP = 1                # k-tiles per exp group (psum group tile)
HORDER = (0, 2, 1, 3)   # even heads first: their avf partition-shift DMAs
                        # start earlier and hide under the odd heads' attn

TRACE = False
TRACE_KW = {}
LAST_RESULT = None
_CACHE = {}


def _chunk(lst, n):
    return [lst[i : i + n] for i in range(0, len(lst), n)]


def _build(ktiles, mk_w, zero_bias, num_devices=N_CORES, sim_init=False):
    """Emit the bass program. ktiles[j] = [(t, lo, tri), ...] computed
    k-tiles for q-chunk j (see _classify_mask)."""
    exp_scale = 0.125
    nc = bacc.Bacc(
        "TRN2", target_bir_lowering=False, debug=False, num_devices=num_devices
    )
    xq = nc.dram_tensor("xq", [NSC, 128, DK, SC], BF16, kind="ExternalInput").ap()
    xk = nc.dram_tensor("xk", [NSC, 128, DK, SC], BF16, kind="ExternalInput").ap()
    xv = nc.dram_tensor("xv", [NSC, 128, DK, SC], BF16, kind="ExternalInput").ap()
    wq = nc.dram_tensor("wq", [128, DK, OG], BF16, kind="ExternalInput").ap()
    wk = nc.dram_tensor("wk", [128, DK, OG], BF16, kind="ExternalInput").ap()
    wv = nc.dram_tensor("wv", [128, DK, OG], BF16, kind="ExternalInput").ap()
    wd = nc.dram_tensor("wd", [128, 2, D_MODEL], BF16, kind="ExternalInput").ap()
    # biases pre-masked per (parity, oc); only used when nonzero
    pb_q = nc.dram_tensor("pb_q", [128, 2, 2], F32, kind="ExternalInput").ap()
    pb_k = nc.dram_tensor("pb_k", [128, 2, 2], F32, kind="ExternalInput").ap()
    mk = nc.dram_tensor("mk", [128, mk_w], BF16, kind="ExternalInput").ap()
    outp = nc.dram_tensor("outp", [S, D_MODEL], BF16, kind="ExternalOutput").ap()

    Exp = mybir.ActivationFunctionType.Exp

    with tile.TileContext(nc) as tc:
        with (
            tc.tile_pool(name="singles", bufs=1) as singles,
            tc.tile_pool(name="xin", bufs=6) as xin,
            tc.tile_pool(name="exps", bufs=4) as exps,
            tc.tile_pool(name="small", bufs=4) as small,
            tc.tile_pool(name="bcastp", bufs=4) as bcastp,
            tc.tile_pool(name="ost", bufs=3) as ost,
            # PSUM: scores groups (2 banks x 2 bufs) + shared pool for
            # projections / AV accumulation / dense (1 bank x 4 bufs)
            tc.tile_pool(name="psc", bufs=4, space="PSUM") as psc,
            tc.tile_pool(name="ppav", bufs=4, space="PSUM") as ppav,
        ):
            wq_sb = singles.tile([128, DK, OG], BF16)
            nc.sync.dma_start(wq_sb[:], wq)
            # per-partition parity masks ([1]*64+[0]*64 and its complement):
            # multiplying the full-height psum by a mask column writes the
            # head data AND its zero padding in one op. Built by four tiny
            # memsets so no DMA gates the chunk-0 casts.
            pm_sb = singles.tile([128, 2], F32)
            nc.vector.memset(pm_sb[0:64, 0:1], 1.0)
            nc.vector.memset(pm_sb[64:128, 0:1], 0.0)
            nc.vector.memset(pm_sb[0:64, 1:2], 0.0)
            nc.vector.memset(pm_sb[64:128, 1:2], 1.0)
            if not zero_bias:
                pbq_sb = singles.tile([128, 2, 2], F32)
                nc.sync.dma_start(pbq_sb[:], pb_q)
                pbk_sb = singles.tile([128, 2, 2], F32)
                nc.sync.dma_start(pbk_sb[:], pb_k)
            else:
                pbq_sb = pbk_sb = None
            wd_sb = singles.tile([128, 2, D_MODEL], BF16)  # loaded after sc=0 DMAs

            # per-head layouts, zero-padded to K=128 (keeps the PE's HAM
            # activity monitor warm; K=64 streams never unthrottle).
            # head h occupies d-rows [(h%2)*64, (h%2)*64+64); rest zeros.
            qt = singles.tile([128, HPG, S], BF16)
            kt_ = singles.tile([128, HPG, S], BF16)
            # [p = k%128, ktile, head, ones col + 63 zeros + 64 v cols]
            vh1 = singles.tile([128, NKT, HPG, 128], BF16)
            avb = singles.tile([128, 2, S], BF16)   # normalized av^T

            # vh1 cols 1:64 are never initialized: they only feed psum rows
            # 1:64 of the AV output, which nothing reads. qt/kt padding is
            # written by the parity-masked projection casts, so the only
            # startup init is this tiny ones-column memset.
            nc.gpsimd.memset(vh1[:, :, :, 0:1], 1.0)
            if sim_init:
                # CoreSim rejects reads of uninitialized SBUF; on hardware
                # these columns only feed psum rows nothing reads
                nc.gpsimd.memset(vh1[:, :, :, 1:64], 0.0)

            x_tiles = {}

            def emit_chunk_dmas(sc, keys=("q", "k", "v")):
                for key in keys:
                    src = {"q": xq, "k": xk, "v": xv}[key]
                    x_sb = xin.tile([128, DK, SC], BF16, tag="xin")
                    # two half-DMAs so the first projection matmuls start
                    # as soon as the dk 0:4 half lands
                    nc.sync.dma_start(x_sb[:, 0 : DK // 2, :], src[sc, :, 0 : DK // 2])
                    nc.sync.dma_start(x_sb[:, DK // 2 :, :], src[sc, :, DK // 2 :])
                    x_tiles[(sc, key)] = x_sb

            def emit_qk_piece(sc, which, oc):
                """One oc-half (128 output cols = 2 head-halves) of the q or
                k projection for s-chunk sc, written into qt/kt_."""
                x_sb = x_tiles[(sc, which)]
                w_sb = wq_sb if which == "q" else wk_sb
                dst = qt if which == "q" else kt_
                ssl = slice(sc * SC, (sc + 1) * SC)
                ps = ppav.tile([128, SC], F32, tag="ppav")
                for dk in range(DK):
                    nc.tensor.matmul(
                        ps[:],
                        lhsT=w_sb[:, dk, oc * 128 : (oc + 1) * 128],
                        rhs=x_sb[:, dk, :],
                        start=(dk == 0),
                        stop=(dk == DK - 1),
                    )
                pb_sb = pbq_sb if which == "q" else pbk_sb
                for half in range(2):
                    ch = 2 * oc + half
                    if zero_bias:
                        nc.vector.tensor_scalar(
                            out=dst[:, ch, ssl],
                            in0=ps[:],
                            scalar1=pm_sb[:, half : half + 1],
                            scalar2=None,
                            op0=mybir.AluOpType.mult,
                        )
                    else:
                        nc.vector.tensor_scalar(
                            out=dst[:, ch, ssl],
                            in0=ps[:],
                            scalar1=pm_sb[:, half : half + 1],
                            scalar2=pb_sb[:, half, oc : oc + 1],
                            op0=mybir.AluOpType.mult,
                            op1=mybir.AluOpType.add,
                        )

            def emit_v_piece(sc, sth):
                """One 128-row k-tile of the v projection -> vh1."""
                x_sb = x_tiles[(sc, "v")]
                st = sc * (SC // KT) + sth
                ps = ppav.tile([128, SC], F32, tag="ppav")
                ksl = slice(sth * KT, (sth + 1) * KT)
                for dk in range(DK):
                    nc.tensor.matmul(
                        ps[:, :OG],
                        lhsT=x_sb[:, dk, ksl],
                        rhs=wv_sb[:, dk, :],
                        start=(dk == 0),
                        stop=(dk == DK - 1),
                    )
                nc.vector.tensor_copy(
                    out=vh1[:, st, :, 64:128],
                    in_=ps[:, :OG].rearrange("p (h d) -> p h d", d=DEPTH),
                )

            pending_ot = {}

            def emit_dense_piece(st, oc):
                """Half (512 out cols) of a 128-row dense output block."""
                if oc == 0:
                    pending_ot[st] = ost.tile(
                        [128, D_MODEL], BF16, tag="ostage", name=f"ot{st}"
                    )
                ot = pending_ot[st]
                ps = ppav.tile([128, SC], F32, tag="ppav")
                for co in range(2):
                    nc.tensor.matmul(
                        ps[:],
                        lhsT=avb[:, co, st * 128 : (st + 1) * 128],
                        rhs=wd_sb[:, co, oc * 512 : (oc + 1) * 512],
                        start=(co == 0),
                        stop=(co == 1),
                    )
                if oc == 0:
                    nc.vector.tensor_copy(
                        out=ot[:, oc * 512 : (oc + 1) * 512], in_=ps[:]
                    )
                else:
                    nc.vector.tensor_copy(
                        out=ot[:, oc * 512 : (oc + 1) * 512], in_=ps[:]
                    )
                    nc.sync.dma_start(outp[st * 128 : (st + 1) * 128, :], ot[:])
                    del pending_ot[st]

            def emit_attn(h, j, fillers, spacing=1, gctr=None):
                """Causal attention for head h, q-chunk j, popping one filler
                every `spacing` score/exp/AV groups so the independent PE
                work lasts through the whole chunk (late groups are where
                exp throughput limits the pipeline)."""
                if gctr is None:
                    gctr = [0]
                odd = h % 2
                ch = h // 2
                jsl = slice(j * QC, (j + 1) * QC)
                tiles = ktiles[j]
                first, last = tiles[0][0], tiles[-1][0]
                ps_av = ppav.tile([128, QC], F32, tag="ppav")
                if len(tiles) > 1:
                    # 1-tile first group primes the scores->exp->av pipeline
                    groups = [tiles[:1]] + _chunk(tiles[1:], EGRP)
                else:
                    groups = [tiles]
                for grp in groups:
                    ps_g = psc.tile([128, EGRP, QC], F32, tag="psc")
                    for r, (t, lo, runs) in enumerate(grp):
                        # cols [0, lo*128) are fully masked: never computed,
                        # never read by the av matmul below
                        nc.tensor.matmul(
                            ps_g[:, r, lo * 128 :],
                            lhsT=kt_[:, h, t * KT : (t + 1) * KT],
                            rhs=qt[:, h, j * QC + lo * 128 : (j + 1) * QC],
                            start=True,
                            stop=True,
                        )
                    ex = exps.tile([128, EGRP, QC], BF16, tag="exps")
                    if all(lo == 0 for (t, lo, runs) in grp):
                        nc.scalar.activation(
                            out=ex[:, : len(grp), :],
                            in_=ps_g[:, : len(grp), :],
                            func=Exp,
                            scale=0.125,
                        )
                    else:
                        # diagonal tiles: exp only the computed column range
                        for r, (t, lo, runs) in enumerate(grp):
                            nc.scalar.activation(
                                out=ex[:, r, lo * 128 :],
                                in_=ps_g[:, r, lo * 128 :],
                                func=Exp,
                                scale=0.125,
                            )
                    for r, (t, lo, runs) in enumerate(grp):
                        for i0, w, off in runs:
                            nc.vector.tensor_mul(
                                ex[:, r, i0 * 128 : (i0 + w) * 128],
                                ex[:, r, i0 * 128 : (i0 + w) * 128],
                                mk_sb[:, off : off + w * 128],
                            )
                    for r, (t, lo, runs) in enumerate(grp):
                        nc.tensor.matmul(
                            ps_av[:, lo * 128 :],
                            lhsT=vh1[:, t, h, :],
                            rhs=ex[:, r, lo * 128 :],
                            start=(t == first),
                            stop=(t == last),
                        )
                    gctr[0] += 1
                    if fillers and gctr[0] % spacing == 0:
                        fillers.pop(0)()
                # softmax denominator: ones-column -> psum PARTITION 0 ->
                # reciprocal straight out of psum -> gpsimd partition
                # broadcast. No DMA, no staging copy.
                rec0 = small.tile([1, QC], F32, tag="rec0")
                nc.vector.reciprocal_approx_fast(rec0[:], ps_av[0:1, :])
                bc = bcastp.tile([128, QC], F32, tag="bc")
                nc.gpsimd.partition_broadcast(bc[:], rec0[:])
                # av rows live on psum partitions 64:128: normalize straight
                # out of psum into bf16; even heads bounce the normalized
                # bf16 tile through a DMA partition shift into rows 0:64
                if odd:
                    nc.vector.tensor_mul(
                        avb[64:128, ch, jsl], ps_av[64:128, :], bc[64:128, :]
                    )
                else:
                    tmp = bcastp.tile([64, QC], BF16, tag="avtmp")
                    nc.vector.tensor_mul(
                        tmp[:], ps_av[64:128, :], bc[64:128, :]
                    )
                    nc.sync.dma_start(avb[0:64, ch, jsl], tmp[:])

            # ---- interleaved emission ----
            scratch = singles.tile([128, 512], BF16)
            nc.vector.memset(scratch[:], 0.0)
            pw = ppav.tile([128, 512], F32, tag="ppav", name="pwarm")
            for i in range(8):
                nc.tensor.matmul(
                    pw[:],
                    lhsT=scratch[:, 0:128],
                    rhs=scratch[:],
                    start=(i == 0),
                    stop=(i == 7),
                )
            # first-use-ordered DMA issue: xq + wq go first so the first
            # projection matmuls start as early as possible
            emit_chunk_dmas(0, keys=("q",))
            emit_qk_piece(0, "q", 0)
            emit_qk_piece(0, "q", 1)
            wk_sb = singles.tile([128, DK, OG], BF16)
            nc.sync.dma_start(wk_sb[:], wk)
            emit_chunk_dmas(0, keys=("k",))
            emit_qk_piece(0, "k", 0)
            emit_qk_piece(0, "k", 1)
            wv_sb = singles.tile([128, DK, OG], BF16)
            nc.sync.dma_start(wv_sb[:], wv)
            emit_chunk_dmas(0, keys=("v",))
            mk_sb = singles.tile([128, mk_w], BF16)
            nc.sync.dma_start(mk_sb[:], mk)
            for sth in range(SC // KT):
                emit_v_piece(0, sth)
            nc.sync.dma_start(wd_sb[:], wd)  # dense-weight prefetch

            carry = []
            for sc in range(NSC):
                fillers = []
                if sc + 1 < NSC:
                    emit_chunk_dmas(sc + 1)
                    for oc in range(2):
                        fillers.append(
                            lambda sc=sc, oc=oc: emit_qk_piece(sc + 1, "q", oc)
                        )
                        fillers.append(
                            lambda sc=sc, oc=oc: emit_qk_piece(sc + 1, "k", oc)
                        )
                    for sth in range(SC // KT):
                        fillers.append(
                            lambda sc=sc, sth=sth: emit_v_piece(sc + 1, sth)
                        )
                dense = [
                    (lambda st=st, oc=oc: emit_dense_piece(st, oc))
                    for st in range((sc - 1) * 4, sc * 4) if sc >= 1
                    for oc in range(2)
                ]
                # dense pieces have no deadline: defer half of each chunk's
                # so the final chunk (which has no projection fillers left)
                # still has independent PE work between attention groups
                fillers = fillers + carry + (dense[:4] if sc + 1 < NSC else dense)
                carry = dense[4:] if sc + 1 < NSC else []
                ngroups = 4 * max(1, len(ktiles[sc]))
                spacing = max(1, ngroups // max(1, len(fillers)))
                gctr = [0]
                for h in HORDER:
                    emit_attn(h, sc, fillers, spacing, gctr)
                if sc + 1 == NSC:
                    for f in fillers:
                        f()
                else:
                    # leftover projection pieces must land before the next
                    # chunk's attention; dense leftovers join the carry
                    for f in fillers:
                        f()
            for st in range((NSC - 1) * 4, NKT):
                for oc in range(2):
                    emit_dense_piece(st, oc)

    nc.compile()
    return nc


def _classify_mask(mask):
    """Classify 128(k) x 128(q) score blocks from the actual mask contents.

    Returns (ktiles, mk_arr):
      ktiles[j]: list of (t, lo, runs) per computed k-tile for q-chunk j:
        lo: first kept 128-col block within the 512-wide q-chunk (cols
            [0, lo*128) are fully masked and simply never computed/read)
        runs: [(i0, w, off), ...] maximal runs of w consecutive 128-col
            blocks needing a factor multiply; their concatenated factor
            tile lives at mk_arr[:, off : off + w*128]
      mk_arr: [128, W] bf16 multiplicative factors exp(-1e9*m/8)
    """
    m2 = np.asarray(mask, dtype=np.float32).reshape(S, S)
    F = np.exp(m2 * np.float32(-1.25e8))  # exp(-1e9*m/8); 0/1 masks -> 0/1
    if (F.max(axis=1) == 0.0).any():
        raise RuntimeError("mask has fully-masked rows; unsupported")
    blocks = F.reshape(NKT, 128, NKT, 128)  # [qi, qr, t, kr]
    kept = (blocks == 1.0).all(axis=(1, 3))  # [qi, t]
    skip = (blocks == 0.0).all(axis=(1, 3))

    NB = QC // 128  # 128-col blocks per q-chunk
    ktiles = []
    uniq = {}
    chunks = []
    width = [0]

    def run_off(j, t, i0, w):
        fb = np.ascontiguousarray(
            F[(j * NB + i0) * 128 : (j * NB + i0 + w) * 128,
              t * KT : (t + 1) * KT].T
        ).astype(NPBF16)  # [128 k, w*128 q]
        key = fb.tobytes()
        if key not in uniq:
            uniq[key] = width[0]
            chunks.append(fb)
            width[0] += w * 128
        return uniq[key]

    for j in range(NQC):
        qis = list(range(j * NB, (j + 1) * NB))
        tl = []
        for t in range(NKT):
            stats = [
                "k" if kept[qi, t] else ("s" if skip[qi, t] else "m")
                for qi in qis
            ]
            if all(s == "s" for s in stats):
                continue
            lo = next(i for i, s in enumerate(stats) if s != "s")
            need = [i for i in range(lo, NB) if stats[i] != "k"]
            tl.append((t, lo, need))
        if not tl:
            raise RuntimeError("q-chunk with no kept k-tiles; unsupported")
        # the first computed tile must span the full chunk (av 'start' MM)
        if tl[0][1] != 0:
            t0, lo0, need0 = tl[0]
            tl[0] = (t0, 0, sorted(set(need0) | set(range(lo0))))
        # merge consecutive blocks into single wide factor multiplies
        tl2 = []
        for t, lo, need in tl:
            runs = []
            for i in need:
                if runs and runs[-1][0] + runs[-1][1] == i:
                    runs[-1] = (runs[-1][0], runs[-1][1] + 1)
                else:
                    runs.append((i, 1))
            tl2.append(
                (t, lo, [(i0, w, run_off(j, t, i0, w)) for i0, w in runs])
            )
        ktiles.append(tl2)
    if not chunks:
        chunks.append(np.ones((128, KT), dtype=NPBF16))
        width[0] = KT
    mk_arr = np.ascontiguousarray(np.concatenate(chunks, axis=1))
    return ktiles, mk_arr


def _xt_prep(x):
    """[S, D] f32 -> [NSC, 128, DK, SC] bf16, d-major, contiguous S-quarters."""
    xt = x.T.astype(NPBF16)  # [D, S]
    a = xt.reshape(DK, 128, NSC, SC).transpose(2, 1, 0, 3)
    return np.ascontiguousarray(a)


def kernel(v, k, q, mask, wq_w, wq_b, wk_w, wk_b, wv_w, wv_b, dense_w, dense_b):
    global LAST_RESULT
    v = np.asarray(v, dtype=np.float32)
    k = np.asarray(k, dtype=np.float32)
    q = np.asarray(q, dtype=np.float32)
    mask = np.asarray(mask, dtype=np.float32)
    wq_w = np.asarray(wq_w, dtype=np.float32)
    wk_w = np.asarray(wk_w, dtype=np.float32)
    wv_w = np.asarray(wv_w, dtype=np.float32)
    dense_w = np.asarray(dense_w, dtype=np.float32)
    wq_b = np.asarray(wq_b, dtype=np.float32)
    wk_b = np.asarray(wk_b, dtype=np.float32)
    wv_b = np.asarray(wv_b, dtype=np.float32)
    dense_b = np.asarray(dense_b, dtype=np.float32)

    ktiles, mk_arr = _classify_mask(mask)
    zero_bias = not (np.any(wq_b) or np.any(wk_b))
    key = (
        tuple(tuple((t, lo, tuple(rn)) for t, lo, rn in tl) for tl in ktiles),
        mk_arr.shape[1],
        zero_bias,
    )
    if key not in _CACHE:
        _CACHE[key] = _build(ktiles, mk_arr.shape[1], zero_bias)
    nc = _CACHE[key]

    # per-batch inputs (shared by the 4 cores of each batch)
    xq_b = [_xt_prep(q[b]) for b in range(B)]
    xk_b = [_xt_prep(k[b]) for b in range(B)]
    xv_b = [_xt_prep(v[b]) for b in range(B)]

    # per-group weights
    def wslice(w, g):
        ws = w[g * OG : (g + 1) * OG, :].T.astype(NPBF16)  # [D, OG]
        return np.ascontiguousarray(ws.reshape(DK, 128, OG).transpose(1, 0, 2))

    def bslice(b_, g):
        # [128, 2(parity), 2(oc)], zero on the complementary parity rows
        a = b_[g * OG : (g + 1) * OG].astype(np.float32).reshape(2, 2, 64)
        out = np.zeros((128, 2, 2), np.float32)
        for oc in range(2):
            out[0:64, 0, oc] = a[oc, 0]
            out[64:128, 1, oc] = a[oc, 1]
        return out

    wq_g = [wslice(wq_w, g) for g in range(GROUPS)]
    wk_g = [wslice(wk_w, g) for g in range(GROUPS)]
    wv_g = [wslice(wv_w, g) for g in range(GROUPS)]
    qb_g = [bslice(wq_b, g) for g in range(GROUPS)]
    kb_g = [bslice(wk_b, g) for g in range(GROUPS)]
    wd_g = []
    for g in range(GROUPS):
        ds = dense_w[:, g * OG : (g + 1) * OG].T.astype(NPBF16)  # [OG, D]
        wd_g.append(
            np.ascontiguousarray(ds.reshape(2, 128, D_MODEL).transpose(1, 0, 2))
        )

    in_maps = []
    for c in range(N_CORES):
        b, g = c // GROUPS, c % GROUPS
        in_maps.append(
            {
                "xq": xq_b[b],
                "xk": xk_b[b],
                "xv": xv_b[b],
                "wq": wq_g[g],
                "wk": wk_g[g],
                "wv": wv_g[g],
                "wd": wd_g[g],
                "mk": mk_arr,
                "pb_q": qb_g[g],
                "pb_k": kb_g[g],
            }
        )

    kw = dict(trace=True, **TRACE_KW) if TRACE else {}
    res = run_bass_kernel_spmd(nc, in_maps, core_ids=list(range(N_CORES)), **kw)
    LAST_RESULT = res

    corr = dense_w @ wv_b + dense_b  # v-bias pushed through dense, + dense bias
    out = np.empty((B, S, D_MODEL), dtype=np.float32)
    for b in range(B):
        acc = np.zeros((S, D_MODEL), dtype=np.float32)
        for g in range(GROUPS):
            acc += res.results[b * GROUPS + g]["outp"].astype(np.float32)
        out[b] = acc + corr
    return out
